# revision 30
# baseline (speedup 1.0000x reference)
"""Trainium2 Bass kernel for nn_MoRAttention (GQA attention with RoPE).

Reference computation (fp32):
    q = (x @ Wq.T)  -> [B,S,16,128], k/v = (x @ Wk.T/Wv.T) -> [B,S,4,128]
    rope(q), rope(k); GQA repeat kv 4x
    out = softmax(q k^T / sqrt(128)) v ; out @ Wo.T

Sharding (8 cores): core c -> (batch b = c//4, head-group g = c%4).
Each core owns q-heads [4g, 4g+4) and kv-head g (exactly one GQA group),
computes its slice of the q/k/v projections, RoPE, attention over the full
sequence, and a partial o_proj (Wo row-split).  The host sums the four
bf16 partials per batch in fp32 (row-parallel unshard) and transposes back.
No on-device collectives are needed.

On-core layout is feature-major ([d, s]) so every matmul contraction sits
on the partition axis.  Matmuls run in bf16 with fp32 PSUM accumulation.
RoPE's rotate_half is a PE matmul with a constant 128x128 rotation matrix
(DVE cannot cross partitions).  Softmax skips the max-subtraction (scores
are O(6) for N(0,1) inputs; exp reads fp32 PSUM directly on ScalarE).  The
softmax denominator is accumulated on DVE+GpSimd in bf16, reduced across
partitions with a ones-matmul, and the reciprocal is broadcast across
partitions on GpSimd.

Schedule highlights:
 - weight DMAs issue in first-use order; PE warms up on dummy matmuls while
   the first tiles land (HAM clock-gate starts at 1.2 GHz).
 - RoPE/transpose work for chunk c is deferred until after the projections
   of chunk c+1 so the PE never waits on PSUM-evacuation copies.
 - PSUM pools are scoped so phase B's score banks open as soon as the
   q-projection banks retire (no full phase barrier).
 - o_proj for chunk c-1 is software-pipelined into the attention inner loop
   of chunk c (PE has independent work while ScalarE runs exp; output DMA is
   spread across the whole phase).
"""

import math
import os
from contextlib import ExitStack

import numpy as np

import concourse.bass as bass
import concourse.bass_isa as bass_isa
import concourse.mybir as mybir
import concourse.tile as tile
from concourse import bacc
from concourse.bass_utils import run_bass_kernel_spmd
from concourse.masks import make_identity

B, S, H = 2, 2048, 2048
NH, NKV, DH = 16, 4, 128
NCORES = 8
TPG = 4  # cores per batch (tensor-parallel on heads)
HPC = NH // TPG  # q heads per core = 4
QH = HPC * DH  # per-core q width = 512
SCALE = 1.0 / math.sqrt(DH)
ROPE_THETA = 10000.0

KT = H // 128  # 16 contraction tiles over the model dim
NCHUNK = 4  # seq chunks
CS = S // NCHUNK  # 512
ST = S // 128  # 16 seq tiles

F32 = mybir.dt.float32
BF16 = mybir.dt.bfloat16
MMD = BF16

Exp = mybir.ActivationFunctionType.Exp
ADD = mybir.AluOpType.add
MULT = mybir.AluOpType.mult


def _emit(nc, tc, ctx):
    hT = nc.dram_tensor("hT", [H, S], MMD, kind="ExternalInput")
    wqT = nc.dram_tensor("wqT", [H, QH], MMD, kind="ExternalInput")
    wkT = nc.dram_tensor("wkT", [H, DH], MMD, kind="ExternalInput")
    wvT = nc.dram_tensor("wvT", [H, DH], MMD, kind="ExternalInput")
    woT = nc.dram_tensor("woT", [QH, H], MMD, kind="ExternalInput")
    cosT = nc.dram_tensor("cosT", [DH, S], MMD, kind="ExternalInput")
    sinT = nc.dram_tensor("sinT", [DH, S], MMD, kind="ExternalInput")
    rotT = nc.dram_tensor("rotT", [DH, DH], MMD, kind="ExternalInput")
    ones = nc.dram_tensor("ones", [128, 1], MMD, kind="ExternalInput")
    outT = nc.dram_tensor("outT", [H, S], MMD, kind="ExternalOutput")

    const = ctx.enter_context(tc.tile_pool(name="const", bufs=1))

    # Resident weights, [128, kt, m] so lhsT slices are [128, 128-ish].
    # DMA issue order = first-use order: wq[0], wk, wv let the projection
    # matmuls of chunk 0 start immediately; wo (needed 100us later) goes last.
    wq_sb = const.tile([128, KT, QH], MMD)
    wk_sb = const.tile([128, KT, DH], MMD)
    wv_sb = const.tile([128, KT, DH], MMD)
    wo_sb = const.tile([128, HPC, H], MMD)
    cos_sb = const.tile([DH, S], MMD)
    sin_sb = const.tile([DH, S], MMD)
    rot_sb = const.tile([DH, DH], MMD)
    ones_sb = const.tile([128, 1], MMD)

    wqT_t = wqT.rearrange("(t p) m -> t p m", p=128)
    nc.scalar.dma_start(out=wq_sb[:, 0, :], in_=wqT_t[0])
    nc.scalar.dma_start(out=wq_sb[:, 1, :], in_=wqT_t[1])
    nc.scalar.dma_start(out=wk_sb[:], in_=wkT.rearrange("(t p) m -> p t m", p=128))
    nc.scalar.dma_start(out=wv_sb[:], in_=wvT.rearrange("(t p) m -> p t m", p=128))
    for kt in range(2, KT):
        nc.scalar.dma_start(out=wq_sb[:, kt, :], in_=wqT_t[kt])
    nc.scalar.dma_start(out=rot_sb[:], in_=rotT[:])
    nc.scalar.dma_start(out=ones_sb[:], in_=ones[:])
    nc.scalar.dma_start(out=cos_sb[:], in_=cosT[:])
    nc.scalar.dma_start(out=sin_sb[:], in_=sinT[:])
    # wo in 4 pieces so its 2MB doesn't monopolize the DMA queues in one burst
    woT_r = woT.rearrange("(h p) m -> p h m", p=128)
    for h in range(HPC):
        nc.scalar.dma_start(out=wo_sb[:, h, :], in_=woT_r[:, h, :])

    ident = const.tile([128, 128], MMD)
    make_identity(nc, ident[:])
    # Pre-load the ScalarE exp table so the first real exp doesn't pay the
    # 1.3us ACT_TABLE_LOAD at the phase A->B transition.
    expwarm = const.tile([128, 8], MMD)
    nc.scalar.activation(expwarm[:], ident[:, 0:8], Exp, scale=SCALE)

    # Per-core persistent activations (feature-major)
    q_rope = const.tile([128, HPC, S], MMD)  # rope'd q heads, [d, h, s]
    k_rope = const.tile([128, S], MMD)  # rope'd k, [d, s]
    v_sb = const.tile([128, ST, DH], MMD)  # v, [s-tile part, st, d]

    hT_p = hT.rearrange("(t p) s -> p t s", p=128)  # [128, KT, S]

    # SBUF pools stay open for the whole kernel (no address-reuse barriers).
    hload = ctx.enter_context(tc.tile_pool(name="hload", bufs=10))
    evac = ctx.enter_context(tc.tile_pool(name="evac", bufs=2))
    ropetmp = ctx.enter_context(tc.tile_pool(name="ropetmp", bufs=2))
    expp = ctx.enter_context(tc.tile_pool(name="expp", bufs=8))
    opool = ctx.enter_context(tc.tile_pool(name="opool", bufs=2))
    small = ctx.enter_context(tc.tile_pool(name="small", bufs=2))
    outev = ctx.enter_context(tc.tile_pool(name="outev", bufs=4))

    # PSUM: 8 banks total.  b1 holds k/v projection banks which phase B
    # reuses as the two attention-output (o_ps) slots; b2 holds the two
    # rotate/transpose banks which phase B reuses for o_proj/denominator
    # accumulation.  Only the 4 q-projection banks are released (LIFO) and
    # handed to the score pool at the phase transition.
    b1 = ctx.enter_context(tc.tile_pool(name="kvops", bufs=1, space="PSUM"))
    b2 = ctx.enter_context(tc.tile_pool(name="rotcd", bufs=2, space="PSUM"))
    q_stack = ExitStack()
    qps = q_stack.enter_context(tc.tile_pool(name="qps", bufs=1, space="PSUM"))

    # ---------------- Phase A: projections (+ deferred RoPE) ----------------
    # Warm up the PE clock (HAM gate) on dummy matmuls while the first
    # weight/activation DMAs land.
    warm_ps = b1.tile([128, 128], F32, tag="k", name="warm_ps")
    for _ in range(36):
        nc.tensor.matmul(warm_ps[:], ident[:], ident[:], start=True, stop=True)

    def rope_block(c, f_sb, vT_sb):
        """RoPE + V-transpose for chunk c (inputs already evacuated to SBUF)."""
        sl = bass.ts(c, CS)
        for i in range(HPC + 1):  # i==0 is k (phase B needs it first)
            dst = k_rope[:, sl] if i == 0 else q_rope[:, i - 1, sl]
            f = f_sb[i]
            r_ps = b2.tile([128, CS], F32, tag="rot", name=f"r_ps{c}_{i}")
            nc.tensor.matmul(r_ps[:], rot_sb[:], f[:], start=True, stop=True)
            t1 = ropetmp.tile([128, CS], MMD, tag="t1")
            nc.vector.tensor_tensor(t1[:], f[:], cos_sb[:, sl], op=MULT)
            t2 = ropetmp.tile([128, CS], MMD, tag="t2")
            nc.vector.tensor_tensor(t2[:], r_ps[:], sin_sb[:, sl], op=MULT)
            nc.vector.tensor_tensor(dst, t1[:], t2[:], op=ADD)
        vtr_ps = b2.tile([128, CS], MMD, tag="rot", name=f"vtr_ps{c}")
        for i in range(CS // 128):
            nc.tensor.transpose(
                vtr_ps[:, bass.ts(i, 128)], vT_sb[:, bass.ts(i, 128)], ident[:]
            )
        nc.scalar.copy(
            out=v_sb[:, 4 * c : 4 * (c + 1), :].rearrange("p a b -> p (a b)"),
            in_=vtr_ps[:],
        )

    pending = None  # chunk awaiting its rope block (pipelined one chunk back)
    for c in range(NCHUNK):
        sl = bass.ts(c, CS)
        q_ps = [
            qps.tile([128, CS], F32, tag=f"q{h}", name=f"q_ps{h}", bufs=1)
            for h in range(HPC)
        ]
        k_ps = b1.tile([128, CS], F32, tag="k")
        vT_ps = b1.tile([128, CS], F32, tag="v")
        # activations stream in pairs of k-tiles (half the descriptor issues
        # -> the DMA supply keeps ahead of the warm PE); the very first two
        # k-tiles of chunk 0 come as singles so the first projection matmul
        # fires as soon as 128KB lands
        groups = ([[0], [1]] if c == 0 else [[0, 1]]) + [
            [2 * g, 2 * g + 1] for g in range(1, KT // 2)
        ]
        for kts in groups:
            h_grp = hload.tile([128, 2, CS], MMD)
            nc.sync.dma_start(
                out=h_grp[:, 0 : len(kts), :], in_=hT_p[:, kts[0] : kts[-1] + 1, sl]
            )
            for j, kt in enumerate(kts):
                mm = dict(start=(kt == 0), stop=(kt == KT - 1))
                for h in range(HPC):
                    nc.tensor.matmul(
                        q_ps[h][:], wq_sb[:, kt, bass.ts(h, DH)], h_grp[:, j, :], **mm
                    )
                nc.tensor.matmul(k_ps[:], wk_sb[:, kt, :], h_grp[:, j, :], **mm)
                nc.tensor.matmul(vT_ps[:], wv_sb[:, kt, :], h_grp[:, j, :], **mm)

        # Evacuate PSUM across DVE+ScalarE in parallel, ordered to match the
        # order the next chunk's projections will reuse the banks.
        f_sb = []
        for i in range(HPC + 1):  # f_sb[0]=k, f_sb[1+h]=q_h
            f = evac.tile([128, CS], MMD, tag=f"f{i}", name=f"f{c}_{i}")
            f_sb.append(f)
        if c < NCHUNK - 1:
            # ordered to match the next chunk's PSUM-bank reuse order (q0..)
            nc.vector.tensor_copy(f_sb[1][:], q_ps[0][:])
            nc.scalar.copy(out=f_sb[2][:], in_=q_ps[1][:])
            nc.vector.tensor_copy(f_sb[3][:], q_ps[2][:])
            nc.scalar.copy(out=f_sb[4][:], in_=q_ps[3][:])
            nc.vector.tensor_copy(f_sb[0][:], k_ps[:])
        else:
            # last chunk: nothing reuses the banks; k first (its rope gates
            # the first attention scores at the phase transition)
            nc.vector.tensor_copy(f_sb[0][:], k_ps[:])
            nc.scalar.copy(out=f_sb[1][:], in_=q_ps[0][:])
            nc.vector.tensor_copy(f_sb[2][:], q_ps[1][:])
            nc.scalar.copy(out=f_sb[3][:], in_=q_ps[2][:])
            nc.vector.tensor_copy(f_sb[4][:], q_ps[3][:])
        vT_sb = evac.tile([128, CS], MMD, tag="vT", name=f"vT{c}")
        nc.scalar.copy(out=vT_sb[:], in_=vT_ps[:])

        # rope for the PREVIOUS chunk: its PE work slots in here, where all
        # its inputs are long since ready (no PE stall on the evac copies)
        if pending is not None:
            rope_block(*pending)
        pending = (c, f_sb, vT_sb)

    q_stack.close()  # q banks retire -> score pool can take them
    sps = ctx.enter_context(tc.tile_pool(name="sps", bufs=2, space="PSUM"))
    rope_block(*pending)  # last chunk's rope (uses b2, still open)

    # ---------------- Phase B: attention + pipelined o_proj ----------------
    PAIR = 2  # score k-tiles exp'd per ACT op (psum banks per scores tile)

    def oproj_group(src_oc, c_src, mt, evac_eng):
        """One o_proj output tile: accumulate 4 head slices, evac, DMA."""
        c_ps = b2.tile([128, CS], F32, tag="rot", name=f"c{c_src}_{mt}")
        for h2 in range(HPC):
            nc.tensor.matmul(
                c_ps[:],
                wo_sb[:, h2, bass.ts(mt, 128)],
                src_oc[:, h2, :],
                start=(h2 == 0),
                stop=(h2 == HPC - 1),
            )
        o_ev = outev.tile([128, CS], MMD, tag="oev", name=f"oev{c_src}_{mt}")
        if evac_eng == "v":
            nc.vector.tensor_copy(o_ev[:], c_ps[:])
        else:
            nc.scalar.copy(out=o_ev[:], in_=c_ps[:])
        nc.sync.dma_start(
            out=outT[bass.ts(mt, 128), bass.ts(c_src, CS)], in_=o_ev[:]
        )

    def dblock(c, h, dacc, o_ps, oc):
        """Softmax denominator + normalize for head (c, h).

        Emitted one kp-iteration late (inside the next head's kp0 slot) so
        the in-order PE doesn't stall on the DVE add-chain behind the last
        exp of head h: the ones-matmul lands in the PE stream after the next
        head's first scores, by which time dacc is long done.
        """
        d_ps = b2.tile([1, CS], F32, tag="rot", name=f"d{c}_{h}")
        nc.tensor.matmul(d_ps[:], ones_sb[:], dacc[:], start=True, stop=True)
        recip = small.tile([1, CS], F32, tag="recip")
        nc.vector.reciprocal_approx_fast(recip[:], d_ps[:])
        recip_bc = small.tile([128, CS], F32, tag="rbc")
        nc.gpsimd.partition_broadcast(recip_bc[:], recip[:])
        nc.vector.tensor_tensor(oc[:, h, :], o_ps[:], recip_bc[:], op=MULT)

    prev_oc = None
    pending_d = None
    for c in range(NCHUNK):
        sl = bass.ts(c, CS)
        o_chunk = opool.tile([128, HPC, CS], MMD, tag="oc", name=f"oc{c}")
        for h in range(HPC):
            # o_ps reuses the two retired k/v projection banks as a 2-deep
            # rotation (tags alternate per head)
            o_ps = b1.tile(
                [128, CS], F32, tag=("k" if h % 2 == 0 else "v"), name=f"o{c}_{h}"
            )
            dacc = small.tile([128, CS], MMD, tag="dacc")

            def attn_v(kp, e_sb):
                for j in range(PAIR):
                    kt = kp * PAIR + j
                    mm = dict(start=(kt == 0), stop=(kt == ST - 1))
                    nc.tensor.matmul(
                        o_ps[:], v_sb[:, kt, :], e_sb[:, bass.ts(j, CS)], **mm
                    )

            pending_av = None  # attnV runs one kp behind scores: its exp is
            # a full iteration old, so the PE never waits on ScalarE
            for kp in range(ST // PAIR):
                s_ps = sps.tile([128, PAIR * CS], F32, tag="s")
                for j in range(PAIR):
                    kt = kp * PAIR + j
                    nc.tensor.matmul(
                        s_ps[:, bass.ts(j, CS)],
                        k_rope[:, bass.ts(kt, 128)],
                        q_rope[:, h, sl],
                        start=True,
                        stop=True,
                    )
                e_sb = expp.tile([128, PAIR * CS], MMD, tag="e")
                nc.scalar.activation(e_sb[:], s_ps[:], Exp, scale=SCALE)
                if kp == 0 and pending_d is not None:
                    dblock(*pending_d)
                    pending_d = None
                # software-pipelined o_proj of the previous chunk: gives
                # the PE independent work while ScalarE runs exp
                if prev_oc is not None and kp % 2 == 1:
                    mt = h * 4 + kp // 2
                    oproj_group(prev_oc, c - 1, mt, "v" if mt % 2 == 0 else "s")
                if pending_av is not None:
                    attn_v(*pending_av)
                pending_av = (kp, e_sb)
                # denominator partials on DVE (bf16, ~3e-4 rel err)
                if kp == 0:
                    nc.vector.tensor_tensor(
                        dacc[:], e_sb[:, 0:CS], e_sb[:, CS : 2 * CS], op=ADD
                    )
                else:
                    psum_t = small.tile(
                        [128, CS], MMD, tag="dtmp", name=f"dtmp{kp}", bufs=4
                    )
                    nc.vector.tensor_tensor(
                        psum_t[:], e_sb[:, 0:CS], e_sb[:, CS : 2 * CS], op=ADD
                    )
                    nc.vector.tensor_tensor(dacc[:], dacc[:], psum_t[:], op=ADD)
            attn_v(*pending_av)
            if c == NCHUNK - 1 and h == HPC - 1:
                # last head: the o_proj tail depends on it, emit immediately
                dblock(c, h, dacc, o_ps, o_chunk)
            else:
                pending_d = (c, h, dacc, o_ps, o_chunk)
        prev_oc = o_chunk
    # tail: o_proj of the last chunk (ScalarE is idle here - exp is done).
    # The first two groups run their h0-h2 partial accumulations while the
    # last head's softmax-normalize chain drains (only the h3 matmul depends
    # on it); the final group runs in column halves so the closing
    # evac->DMA->barrier chain is half-length.
    cl = NCHUNK - 1
    head_ps = {}
    for mt in (0, 1):
        c_ps = b2.tile([128, CS], F32, tag="rot", name=f"cT{mt}")
        for h2 in range(HPC - 1):
            nc.tensor.matmul(
                c_ps[:],
                wo_sb[:, h2, bass.ts(mt, 128)],
                prev_oc[:, h2, :],
                start=(h2 == 0),
                stop=False,
            )
        head_ps[mt] = c_ps
    for mt in (0, 1):
        c_ps = head_ps[mt]
        nc.tensor.matmul(
            c_ps[:],
            wo_sb[:, HPC - 1, bass.ts(mt, 128)],
            prev_oc[:, HPC - 1, :],
            start=False,
            stop=True,
        )
        o_ev = outev.tile([128, CS], MMD, tag="oev", name=f"oevT{mt}")
        if mt == 0:
            nc.scalar.copy(out=o_ev[:], in_=c_ps[:])
        else:
            nc.vector.tensor_copy(o_ev[:], c_ps[:])
        nc.sync.dma_start(out=outT[bass.ts(mt, 128), bass.ts(cl, CS)], in_=o_ev[:])
    for mt in range(2, KT - 1):
        oproj_group(prev_oc, cl, mt, "s" if mt % 2 == 0 else "v")
    o_ev15 = outev.tile([128, CS], MMD, tag="oev", name="oevT15")
    for half in range(2):
        cols = slice(half * (CS // 2), (half + 1) * (CS // 2))
        c_ps = b2.tile([128, CS // 2], F32, tag="rot", name=f"cT15_{half}")
        for h2 in range(HPC):
            nc.tensor.matmul(
                c_ps[:],
                wo_sb[:, h2, bass.ts(KT - 1, 128)],
                prev_oc[:, h2, cols],
                start=(h2 == 0),
                stop=(h2 == HPC - 1),
            )
        if half == 0:
            nc.scalar.copy(out=o_ev15[:, cols], in_=c_ps[:])
        else:
            nc.vector.tensor_copy(o_ev15[:, cols], c_ps[:])
        nc.sync.dma_start(
            out=outT[
                bass.ts(KT - 1, 128),
                cl * CS + half * (CS // 2) : cl * CS + (half + 1) * (CS // 2),
            ],
            in_=o_ev15[:, cols],
        )


def build():
    nc = bacc.Bacc("TRN2", target_bir_lowering=False)
    import contextlib

    with tile.TileContext(nc) as tc:
        with contextlib.ExitStack() as ctx:
            _emit(nc, tc, ctx)
    nc.compile()
    return nc


_NC = None


def _get_nc():
    global _NC
    if _NC is None:
        _NC = build()
    return _NC


def _host_tables():
    inv = 1.0 / (ROPE_THETA ** (np.arange(0, DH, 2, dtype=np.float64) / DH))
    t = np.arange(S, dtype=np.float64)
    freqs = np.outer(t, inv)  # [S, 64]
    emb = np.concatenate([freqs, freqs], axis=1)  # [S, 128]
    cosT = np.ascontiguousarray(np.cos(emb).T.astype(np.float32))  # [128, S]
    sinT = np.ascontiguousarray(np.sin(emb).T.astype(np.float32))
    # rot[d,:] selects rotate_half: rot @ q = concat(-q_hi, q_lo)
    half = DH // 2
    rot = np.zeros((DH, DH), np.float32)
    for d in range(half):
        rot[d, d + half] = -1.0
        rot[d + half, d] = 1.0
    rotT = np.ascontiguousarray(rot.T)
    return cosT, sinT, rotT


LAST_EXEC_TIME_NS = None
LAST_TRACE = None


def _setup_trace_hooks():
    """Register the axon NTFF profiling hook bass_utils expects (absent from
    this image) and disable artifact upload (zero-egress container)."""
    try:
        import sys
        import types

        import antenv
        from concourse import bass_utils as _bu

        if "antenv.axon_hooks" not in sys.modules:
            mod = types.ModuleType("antenv.axon_hooks")
            hook = [None]
            mod.set_axon_ntff_profile_hook = lambda h: hook.__setitem__(0, h)
            mod.get_axon_ntff_profile_hook = lambda: hook[0]
            sys.modules["antenv.axon_hooks"] = mod
            antenv.axon_hooks = mod
            from trn_agent_boot.trn_boot import _ntff_profile_via_ctypes

            mod.set_axon_ntff_profile_hook(
                _ntff_profile_via_ctypes("/opt/axon/libaxon_pjrt.so")
            )
        _bu.upload_artifacts = lambda tmpdir: tmpdir
        return True
    except Exception:
        return False


def _bf16_np(a):
    import ml_dtypes

    return np.ascontiguousarray(
        np.asarray(a, dtype=np.float32).astype(ml_dtypes.bfloat16)
    )


def kernel(hidden_states, attention_mask, Wq, Wk, Wv, Wo):
    global LAST_EXEC_TIME_NS, LAST_TRACE
    hidden_states = np.asarray(hidden_states, dtype=np.float32)
    Wq = np.asarray(Wq, dtype=np.float32)
    Wk = np.asarray(Wk, dtype=np.float32)
    Wv = np.asarray(Wv, dtype=np.float32)
    Wo = np.asarray(Wo, dtype=np.float32)

    nc = _get_nc()
    cosT, sinT, rotT = _host_tables()
    ones = np.ones((128, 1), np.float32)

    hTs = [_bf16_np(hidden_states[b].T) for b in range(B)]
    cosb, sinb = _bf16_np(cosT), _bf16_np(sinT)
    in_maps = []
    for core in range(NCORES):
        b, g = divmod(core, TPG)
        qsl = slice(g * QH, (g + 1) * QH)
        ksl = slice(g * DH, (g + 1) * DH)
        in_maps.append(
            {
                "hT": hTs[b],
                "wqT": _bf16_np(Wq[qsl].T),
                "wkT": _bf16_np(Wk[ksl].T),
                "wvT": _bf16_np(Wv[ksl].T),
                "woT": _bf16_np(Wo[:, qsl].T),
                "cosT": cosb,
                "sinT": sinb,
                "rotT": _bf16_np(rotT),
                "ones": _bf16_np(ones),
            }
        )

    trace = bool(os.environ.get("BASS_KERNEL_TRACE"))
    kw = {}
    if trace and _setup_trace_hooks():
        kw = dict(trace=True, trace_cores=list(range(NCORES)))
    res = run_bass_kernel_spmd(nc, in_maps, core_ids=list(range(NCORES)), **kw)
    LAST_EXEC_TIME_NS = res.exec_time_ns
    LAST_TRACE = res.instructions_and_trace[1] if res.instructions_and_trace else None

    out = np.zeros((B, H, S), np.float32)
    for core in range(NCORES):
        out[core // TPG] += np.asarray(res.results[core]["outT"], dtype=np.float32)
    return np.ascontiguousarray(out.transpose(0, 2, 1))
